# revision 15
# baseline (speedup 1.0000x reference)
"""Trainium2 Bass kernel for nn_Bspline_19335942766607.

inputs [16, 25, 2048] f32 -> flow [16, 25, 192, 192, 2] f32.

Math: each of the 400 samples is a 32x32x2 control-point grid, bilinearly
resampled to 192x192 per channel and scaled by -192.  The query grid is
fixed, so per sample and channel this is two constant-matrix products:
    T_c = (-192 * Ay) @ P_c        Ay [192,32] interpolation matrix
    D_c = T_c @ Ax^T               Ax [192,32]

Kernel design (per core, 50 samples; pure data-parallel over 8 cores):
- single-fp16 arithmetic with fp32 PSUM accumulation; exact fp16
  constants (-192*Ay entries are integers; 3*Ax entries are k/64), tt
  carried as tt/3.  End-to-end rel err ~8e-4 (fp16 rounding of p, tt,
  output), far inside the 2e-2 gate.
- OUTPUT IS WRITTEN fp16 (host upcasts to f32 after gather): halves the
  HBM write traffic.
- SUPERPAIR structure (4 samples = 2 pairs per sync unit) keeps the PE
  in long back-to-back bursts (8 matmuls, ~3k columns) so it holds its
  high p-state; per-unit cross-engine round trips are halved:
  * stage-1: ONE matmul per pair via block-diagonal rhs ayd [64, 384]
    ({ayt|0; 0|ayt}): lhsT = host-packed [64 (2 samples' g-rows), 64 (m)]
    pair block, N=384.  Pair A -> tt[0:64] (tile col 0), pair B ->
    tt[64:128] (tile col 64) of one [128, 384] PSUM bank.
  * ACT converts the whole superpair: hl [128, 384] = fp16(tt/3).
  * stage-2: 6 matmuls (pair x stripe k): lhsT = hl[half, k:384:3]
    [64, 128] (stride 3 runs through sample a's cols into sample b's),
    rhs = axt2 (rows 64:128 duplicate 0:64 so pair B can contract on
    array rows 64:128 via tile_position (64, 0)).  Stripe k holds
    pair-row r = 3p + k on partition p, so three stripes concatenate
    into one contiguous [128 x 2304 B] fp16 block per pair.
- each pair's 3 stripes land in ONE 3-bank PSUM tile (stripe k at
  element col 512k, bank-aligned) and leave in ONE strided ACT/DVE
  copy -- fewest PSUM->SBUF ops and cross-engine sync edges, which HW
  measurement (not the cost model) shows is what the composed pipeline
  is actually paying for.  Copies are emitted BEFORE the next unit's
  convert on ACT so the PSUM slot the next PE burst waits on frees
  first.
- output DMA: 4 pairs (2 units) share one SBUF tile, leaving in a
  single ~1.15 MB contiguous DMA alternated between the sync and
  gpsimd DGE rings (ACT stays off DMA duty).
- PSUM: tt double-buffered + 2 triple-stripe slots (8 banks).
- measured (8 axon trn2 cores, For_i-loop contrast): ~33-35 us/exec;
  the prior-session baseline measured ~55-60 us on the same harness.
"""

import sys

if "/opt/trn_rl_repo" not in sys.path:
    sys.path.insert(0, "/opt/trn_rl_repo")

import numpy as np

import concourse.mybir as mybir
from concourse import bacc
from concourse.bass import ds
from concourse.bass_utils import run_bass_kernel_spmd
from concourse.tile import TileContext

F32 = mybir.dt.float32
F16 = mybir.dt.float16

B, T = 16, 25
H, W = 192, 192
G = 32
N_CORES = 8
N_SAMPLES = B * T                   # 400
S_PER_CORE = N_SAMPLES // N_CORES   # 50
FW = 2 * W                          # 384
H2 = 2 * H                          # 384 (pair cols: sample a | sample b)


def _interp_weights(size_out, size_in):
    q = (np.arange(size_out, dtype=np.float32) / np.float32(size_out)) * np.float32(
        size_in - 1
    )
    f = np.clip(np.floor(q), np.float32(0.0), np.float32(size_in - 2))
    idx0 = f.astype(np.int32)
    alpha = np.clip(q - f, np.float32(0.0), np.float32(1.0))
    return idx0, alpha


def _make_constants():
    """ayd [64,384] f16 block-diag {fp16((-192*Ay)^T)|0; 0|same};
    axt2 [128,384] f16 = fp16(3*Ax)^T channel-interleaved, rows 64:128
    duplicating rows 0:64."""
    y0, ay = _interp_weights(H, G)
    x0, ax = _interp_weights(W, G)
    Ay = np.zeros((H, G), dtype=np.float32)
    Ay[np.arange(H), y0] = np.float32(1.0) - ay
    Ay[np.arange(H), y0 + 1] += ay
    Ax = np.zeros((W, G), dtype=np.float32)
    Ax[np.arange(W), x0] = np.float32(1.0) - ax
    Ax[np.arange(W), x0 + 1] += ax
    ayt16 = (np.float32(-H) * Ay).T.astype(np.float16)        # [32, 192]
    ayd = np.zeros((2 * G, H2), dtype=np.float16)
    ayd[0:G, 0:H] = ayt16
    ayd[G : 2 * G, H:H2] = ayt16
    ax3 = (np.float32(3.0) * Ax).T.astype(np.float16)         # [32, 192]
    axt2 = np.zeros((128, FW), dtype=np.float16)
    for c in range(2):
        axt2[c * G : (c + 1) * G, c::2] = ax3
        axt2[64 + c * G : 64 + (c + 1) * G, c::2] = ax3
    return np.ascontiguousarray(ayd), np.ascontiguousarray(axt2)


def build(n_samples=S_PER_CORE, n_reps=1, n_loop=1):
    """Per-core Bass program (SPMD across 8 cores).

    n_reps: python-unrolled repetitions of the whole workload (timing).
    n_loop: hardware-loop (tc.For_i) iterations around those reps --
        keeps the instruction count flat for large timing contrasts.
    """
    assert n_samples % 2 == 0
    npair = n_samples // 2            # 25
    # units: list of pair-tuples; full superpairs then a leftover pair
    units = [(2 * q, 2 * q + 1) for q in range(npair // 2)]
    if npair % 2:
        units.append((npair - 1,))
    nu = len(units)
    nc = bacc.Bacc(None, target_bir_lowering=False, debug=False)
    # pp arrives host-packed [64, npair*64]: pair j at cols 64j:64j+64,
    # partitions 0:32 = sample 2j's g-rows, 32:64 = sample 2j+1's.
    pp_ext = nc.declare_dram_parameter("pp", [2 * G, npair * 2 * G], F16, isOutput=False)
    ayd_ext = nc.declare_dram_parameter("ayd", [2 * G, H2], F16, isOutput=False)
    axt_ext = nc.declare_dram_parameter("axt2", [128, FW], F16, isOutput=False)
    out_ext = nc.declare_dram_parameter(
        "out", [n_samples, H, FW], F16, isOutput=True
    )
    dma_batch = 4  # pairs per output DMA

    with TileContext(nc) as tc:
        with (
            tc.tile_pool(name="const", bufs=1) as cpool,
            tc.tile_pool(name="work", bufs=4) as wpool,
            tc.tile_pool(name="psum", bufs=1, space="PSUM") as pspool,
        ):
            ayd_sb = cpool.tile([2 * G, H2], F16)
            nc.sync.dma_start(out=ayd_sb[:], in_=ayd_ext[:])
            axt_sb = cpool.tile([128, FW], F16)
            nc.sync.dma_start(out=axt_sb[:], in_=axt_ext[:])
            pp_sb = cpool.tile([2 * G, npair * 2 * G], F16)
            nc.sync.dma_start(out=pp_sb[:], in_=pp_ext[:])

            dma_cycle = [nc.sync, nc.gpsimd]

            def rep_body():

                def s1(u):
                    # one block-diag matmul per pair; pair A -> tt[0:64],
                    # pair B -> tt[64:128] (tile col offset 64)
                    tt_ps = pspool.tile([128, H2], F32, tag="tt", bufs=2, name="tt_ps")
                    for half, j in enumerate(units[u]):
                        nc.tensor.matmul(
                            tt_ps[64 * half : 64 * half + 64],
                            pp_sb[:, ds(j * 2 * G, 2 * G)],
                            ayd_sb[:],
                            start=True, stop=True, tile_position=(0, 64 * half),
                        )
                    return tt_ps

                def ctt(u, tt_ps):
                    # hl = fp16(tt/3) on ACT, whole superpair in one op
                    np_half = 64 * len(units[u])
                    hl = wpool.tile([128, H2], F16, tag="hl")
                    nc.scalar.activation(
                        hl[0:np_half], tt_ps[0:np_half],
                        mybir.ActivationFunctionType.Copy, scale=1.0 / 3.0,
                    )
                    return hl

                o_sb_cur = [None]
                dma_i = [0]

                def emit_unit(u, hl):
                    # stage-2 + copies interleaved: per pair, 3 stripe
                    # matmuls; each stripe copied right after its matmul,
                    # alternating DVE / ACT.
                    for half, j in enumerate(units[u]):
                        bi = j % dma_batch
                        if bi == 0:
                            o_sb_cur[0] = wpool.tile(
                                [128, dma_batch * 3 * FW], F16, tag="o_sb",
                                name="o_sb",
                            )
                        o_sb = o_sb_cur[0]
                        off = bi * 3 * FW
                        h0 = 64 * half
                        # one 3-bank PSUM tile per pair: stripe k at element
                        # col 512*k (bank-aligned); evacuated by ONE strided
                        # copy -- fewest cross-engine dependency edges
                        pt = pspool.tile(
                            [128, 3 * 512], F32, tag="pk3", bufs=2, name="pk3"
                        )
                        for k in range(3):
                            nc.tensor.matmul(
                                pt[:, 512 * k : 512 * k + FW],
                                hl[h0 : h0 + 64, k : H2 : 3],
                                axt_sb[h0 : h0 + 64],
                                start=True, stop=True, tile_position=(h0, 0),
                            )
                        src = pt[:].rearrange("p (three x) -> p three x", three=3)[
                            :, :, 0:FW
                        ]
                        dst = o_sb[:, off : off + 3 * FW].rearrange(
                            "p (three x) -> p three x", three=3
                        )
                        # ACT: convert + pair A's copy; DVE: pair B's copy
                        if half == 0 and len(units[u]) > 1:
                            nc.scalar.copy(out=dst, in_=src)
                        else:
                            nc.vector.tensor_copy(out=dst, in_=src)
                        if bi == dma_batch - 1 or j == npair - 1:
                            nb = bi + 1
                            s = 2 * (j - bi)
                            eng = dma_cycle[dma_i[0] % len(dma_cycle)]
                            dma_i[0] += 1
                            # DRAM row (384*jj + 3p + k) <- o_sb[p, ...]
                            dst = (
                                out_ext[s : s + 2 * nb]
                                .rearrange("s h f -> (s h) f")
                                .rearrange("(jj p k) f -> p jj k f", p=128, k=3)
                                .rearrange("p jj k f -> p jj (k f)")
                            )
                            src = o_sb[:, 0 : nb * 3 * FW].rearrange(
                                "p (jj kf) -> p jj kf", jj=nb
                            )
                            eng.dma_start(out=dst, in_=src)

                tt_q = {0: s1(0)}
                hl_q = {0: ctt(0, tt_q.pop(0))}
                if nu > 1:
                    tt_q[1] = s1(1)
                for u in range(nu):
                    # emit_unit first: ACT's copy_A(u) (which frees the PSUM
                    # slot the next PE burst waits on) precedes cvt(u+1),
                    # which isn't consumed until the next iteration.
                    emit_unit(u, hl_q.pop(u))
                    if u + 1 < nu:
                        hl_q[u + 1] = ctt(u + 1, tt_q.pop(u + 1))
                    if u + 2 < nu:
                        tt_q[u + 2] = s1(u + 2)

            if n_loop == 1:
                for _rep in range(n_reps):
                    rep_body()
            else:
                with tc.For_i(0, n_loop, 1):
                    for _rep in range(n_reps):
                        rep_body()
    nc.finalize()
    return nc


_CACHE = {}


def _get_nc(n_reps=1, n_loop=1):
    key = (n_reps, n_loop)
    if key not in _CACHE:
        _CACHE[key] = build(n_reps=n_reps, n_loop=n_loop)
    return _CACHE[key]


def prep_inputs(p_full):
    """p_full [400, 32, 64] f32 (raw [g, (g',c)]) -> per-core in_maps."""
    ayd, axt2 = _make_constants()
    # deinterleave channels: column m = c*32 + g'
    p_d = (
        p_full.reshape(N_SAMPLES, G, G, 2)
        .transpose(0, 1, 3, 2)
        .reshape(N_SAMPLES, G, 2 * G)
    )
    p16 = p_d.astype(np.float16)
    # pack pairs: [npair, 64, 64] with sample 2j on rows 0:32, 2j+1 below
    pp = p16.reshape(N_CORES, S_PER_CORE // 2, 2 * G, 2 * G)
    # host transpose to [core, 64, npair*64] (partition-major)
    pp_t = np.ascontiguousarray(
        pp.transpose(0, 2, 1, 3).reshape(N_CORES, 2 * G, (S_PER_CORE // 2) * 2 * G)
    )
    return [
        {"pp": pp_t[c], "ayd": ayd, "axt2": axt2}
        for c in range(N_CORES)
    ]


def run_on_hw(p_full, n_reps=1):
    """p_full [400, 32, 64] f32 -> out [400, 192, 384] f32."""
    in_maps = prep_inputs(p_full)
    nc = _get_nc(n_reps)
    res = run_bass_kernel_spmd(nc, in_maps, list(range(N_CORES))).results
    out = np.stack([res[c]["out"] for c in range(N_CORES)])
    return out.reshape(N_SAMPLES, H, FW).astype(np.float32)


def kernel(inputs):
    inputs = np.ascontiguousarray(np.asarray(inputs), dtype=np.float32)
    assert inputs.shape == (B, T, 2 * G * G), inputs.shape
    out = run_on_hw(inputs.reshape(N_SAMPLES, G, 2 * G))
    return out.reshape(B, T, H, W, 2)


# revision 16
# speedup vs baseline: 3.0328x; 3.0328x over previous
"""Trainium2 Bass kernel for nn_Bspline_19335942766607.

inputs [16, 25, 2048] f32 -> flow [16, 25, 192, 192, 2] f32.

Math: each of the 400 samples is a 32x32x2 control-point grid, bilinearly
resampled to 192x192 per channel and scaled by -192.  The query grid is
fixed, so per sample and channel this is two constant-matrix products:
    T_c = (-192 * Ay) @ P_c        Ay [192,32] interpolation matrix
    D_c = T_c @ Ax^T               Ax [192,32]

Kernel design (per core, 50 samples; pure data-parallel over 8 cores):
- single-fp16 arithmetic with fp32 PSUM accumulation; exact fp16
  constants (-192*Ay entries are integers; 3*Ax entries are k/64), tt
  carried as tt/3.  End-to-end rel err ~8e-4 (fp16 rounding of p, tt,
  output), far inside the 2e-2 gate.
- OUTPUT IS WRITTEN fp16 (host upcasts to f32 after gather): halves the
  HBM write traffic.
- SUPERPAIR structure (4 samples = 2 pairs per sync unit) keeps the PE
  in long back-to-back bursts (8 matmuls, ~3k columns) so it holds its
  high p-state; per-unit cross-engine round trips are halved:
  * stage-1: ONE matmul per UNIT via block-diagonal rhs ayd [64, 384]
    ({ayt|0; 0|ayt}): lhsT [64, 128] = the two adjacent host-packed
    pair blocks, so pair A lands on out partitions 0:64 and pair B on
    64:128 of one [128, 384] PSUM bank, N=384.
  * convert hl = fp16(tt/3) split ACT (pair A) / DVE (pair B) to
    balance the two PSUM-evacuation engines.
  * stage-2: 6 matmuls (pair x stripe k): lhsT = hl[half, k:384:3]
    [64, 128] (stride 3 runs through sample a's cols into sample b's),
    rhs = axt2 (rows 64:128 duplicate 0:64 so pair B can contract on
    array rows 64:128 via tile_position (64, 0)).  Stripe k holds
    pair-row r = 3p + k on partition p, so three stripes concatenate
    into one contiguous [128 x 2304 B] fp16 block per pair.
- each pair's 3 stripes land in ONE 3-bank PSUM tile (stripe k at
  element col 512k, bank-aligned) and leave in ONE strided ACT/DVE
  copy -- fewest PSUM->SBUF ops and cross-engine sync edges, which HW
  measurement (not the cost model) shows is what the composed pipeline
  is actually paying for.  Copies are emitted BEFORE the next unit's
  convert on ACT so the PSUM slot the next PE burst waits on frees
  first.
- output DMA: 4 pairs (2 units) share one SBUF tile, leaving in a
  single ~1.15 MB contiguous DMA alternated between the sync and
  gpsimd DGE rings (ACT stays off DMA duty).
- PSUM: tt double-buffered + 2 triple-stripe slots (8 banks).
- measured (8 axon trn2 cores, For_i-loop contrast): ~33-35 us/exec;
  the prior-session baseline measured ~55-60 us on the same harness.
  PE is the wall: ~330 ns per matmul intrinsic on HW (dependency-free
  75-long bursts measure the same), x 88 matmuls/rep at the structural
  minimum (PSUM 512-f32 banks force 3 stripes; M=128 maxed).
"""

import sys

if "/opt/trn_rl_repo" not in sys.path:
    sys.path.insert(0, "/opt/trn_rl_repo")

import numpy as np

import concourse.mybir as mybir
from concourse import bacc
from concourse.bass import ds
from concourse.bass_utils import run_bass_kernel_spmd
from concourse.tile import TileContext

F32 = mybir.dt.float32
F16 = mybir.dt.float16

B, T = 16, 25
H, W = 192, 192
G = 32
N_CORES = 8
N_SAMPLES = B * T                   # 400
S_PER_CORE = N_SAMPLES // N_CORES   # 50
FW = 2 * W                          # 384
H2 = 2 * H                          # 384 (pair cols: sample a | sample b)


def _interp_weights(size_out, size_in):
    q = (np.arange(size_out, dtype=np.float32) / np.float32(size_out)) * np.float32(
        size_in - 1
    )
    f = np.clip(np.floor(q), np.float32(0.0), np.float32(size_in - 2))
    idx0 = f.astype(np.int32)
    alpha = np.clip(q - f, np.float32(0.0), np.float32(1.0))
    return idx0, alpha


def _make_constants():
    """ayd [64,384] f16 block-diag {fp16((-192*Ay)^T)|0; 0|same};
    axt2 [128,384] f16 = fp16(3*Ax)^T channel-interleaved, rows 64:128
    duplicating rows 0:64."""
    y0, ay = _interp_weights(H, G)
    x0, ax = _interp_weights(W, G)
    Ay = np.zeros((H, G), dtype=np.float32)
    Ay[np.arange(H), y0] = np.float32(1.0) - ay
    Ay[np.arange(H), y0 + 1] += ay
    Ax = np.zeros((W, G), dtype=np.float32)
    Ax[np.arange(W), x0] = np.float32(1.0) - ax
    Ax[np.arange(W), x0 + 1] += ax
    ayt16 = (np.float32(-H) * Ay).T.astype(np.float16)        # [32, 192]
    ayd = np.zeros((2 * G, H2), dtype=np.float16)
    ayd[0:G, 0:H] = ayt16
    ayd[G : 2 * G, H:H2] = ayt16
    ax3 = (np.float32(3.0) * Ax).T.astype(np.float16)         # [32, 192]
    axt2 = np.zeros((128, FW), dtype=np.float16)
    for c in range(2):
        axt2[c * G : (c + 1) * G, c::2] = ax3
        axt2[64 + c * G : 64 + (c + 1) * G, c::2] = ax3
    return np.ascontiguousarray(ayd), np.ascontiguousarray(axt2)


def build(n_samples=S_PER_CORE, n_reps=1, n_loop=1):
    """Per-core Bass program (SPMD across 8 cores).

    n_reps: python-unrolled repetitions of the whole workload (timing).
    n_loop: hardware-loop (tc.For_i) iterations around those reps --
        keeps the instruction count flat for large timing contrasts.
    """
    assert n_samples % 2 == 0
    npair = n_samples // 2            # 25
    # units: list of pair-tuples; full superpairs then a leftover pair
    units = [(2 * q, 2 * q + 1) for q in range(npair // 2)]
    if npair % 2:
        units.append((npair - 1,))
    nu = len(units)
    nc = bacc.Bacc(None, target_bir_lowering=False, debug=False)
    # pp arrives host-packed [64, npair*64]: pair j at cols 64j:64j+64,
    # partitions 0:32 = sample 2j's g-rows, 32:64 = sample 2j+1's.
    pp_ext = nc.declare_dram_parameter("pp", [2 * G, npair * 2 * G], F16, isOutput=False)
    ayd_ext = nc.declare_dram_parameter("ayd", [2 * G, H2], F16, isOutput=False)
    axt_ext = nc.declare_dram_parameter("axt2", [128, FW], F16, isOutput=False)
    out_ext = nc.declare_dram_parameter(
        "out", [n_samples, H, FW], F16, isOutput=True
    )
    dma_batch = 4  # pairs per output DMA

    with TileContext(nc) as tc:
        with (
            tc.tile_pool(name="const", bufs=1) as cpool,
            tc.tile_pool(name="work", bufs=4) as wpool,
            tc.tile_pool(name="psum", bufs=1, space="PSUM") as pspool,
        ):
            ayd_sb = cpool.tile([2 * G, H2], F16)
            nc.sync.dma_start(out=ayd_sb[:], in_=ayd_ext[:])
            axt_sb = cpool.tile([128, FW], F16)
            nc.sync.dma_start(out=axt_sb[:], in_=axt_ext[:])
            pp_sb = cpool.tile([2 * G, npair * 2 * G], F16)
            nc.sync.dma_start(out=pp_sb[:], in_=pp_ext[:])

            dma_cycle = [nc.sync, nc.gpsimd]

            def rep_body():

                def s1(u):
                    # ONE matmul per unit: the two pair blocks sit side by
                    # side in pp_sb, so lhsT [64, 128] maps pair A -> out
                    # partitions 0:64 and pair B -> 64:128 directly.
                    tt_ps = pspool.tile([128, H2], F32, tag="tt", bufs=2, name="tt_ps")
                    j = units[u][0]
                    m = 64 * len(units[u])
                    nc.tensor.matmul(
                        tt_ps[0:m],
                        pp_sb[:, ds(j * 2 * G, m)],
                        ayd_sb[:],
                        start=True, stop=True, tile_position=(0, 0),
                    )
                    return tt_ps

                def ctt(u, tt_ps):
                    # hl = fp16(tt/3), split across ACT (pair A) and DVE
                    # (pair B) to balance the two PSUM-evacuation engines
                    hl = wpool.tile([128, H2], F16, tag="hl")
                    nc.scalar.activation(
                        hl[0:64], tt_ps[0:64],
                        mybir.ActivationFunctionType.Copy, scale=1.0 / 3.0,
                    )
                    if len(units[u]) > 1:
                        nc.vector.tensor_scalar_mul(
                            hl[64:128], tt_ps[64:128], 1.0 / 3.0
                        )
                    return hl

                o_sb_cur = [None]
                dma_i = [0]

                def emit_unit(u, hl):
                    # stage-2 + copies interleaved: per pair, 3 stripe
                    # matmuls; each stripe copied right after its matmul,
                    # alternating DVE / ACT.
                    for half, j in enumerate(units[u]):
                        bi = j % dma_batch
                        if bi == 0:
                            o_sb_cur[0] = wpool.tile(
                                [128, dma_batch * 3 * FW], F16, tag="o_sb",
                                name="o_sb",
                            )
                        o_sb = o_sb_cur[0]
                        off = bi * 3 * FW
                        h0 = 64 * half
                        # one 3-bank PSUM tile per pair: stripe k at element
                        # col 512*k (bank-aligned); evacuated by ONE strided
                        # copy -- fewest cross-engine dependency edges
                        pt = pspool.tile(
                            [128, 3 * 512], F32, tag="pk3", bufs=2, name="pk3"
                        )
                        for k in range(3):
                            nc.tensor.matmul(
                                pt[:, 512 * k : 512 * k + FW],
                                hl[h0 : h0 + 64, k : H2 : 3],
                                axt_sb[h0 : h0 + 64],
                                start=True, stop=True, tile_position=(h0, 0),
                            )
                        src = pt[:].rearrange("p (three x) -> p three x", three=3)[
                            :, :, 0:FW
                        ]
                        dst = o_sb[:, off : off + 3 * FW].rearrange(
                            "p (three x) -> p three x", three=3
                        )
                        # ACT: convert + pair A's copy; DVE: pair B's copy
                        if half == 0 and len(units[u]) > 1:
                            nc.scalar.copy(out=dst, in_=src)
                        else:
                            nc.vector.tensor_copy(out=dst, in_=src)
                        if bi == dma_batch - 1 or j == npair - 1:
                            nb = bi + 1
                            s = 2 * (j - bi)
                            eng = dma_cycle[dma_i[0] % len(dma_cycle)]
                            dma_i[0] += 1
                            # DRAM row (384*jj + 3p + k) <- o_sb[p, ...]
                            dst = (
                                out_ext[s : s + 2 * nb]
                                .rearrange("s h f -> (s h) f")
                                .rearrange("(jj p k) f -> p jj k f", p=128, k=3)
                                .rearrange("p jj k f -> p jj (k f)")
                            )
                            src = o_sb[:, 0 : nb * 3 * FW].rearrange(
                                "p (jj kf) -> p jj kf", jj=nb
                            )
                            eng.dma_start(out=dst, in_=src)

                tt_q = {0: s1(0)}
                hl_q = {0: ctt(0, tt_q.pop(0))}
                if nu > 1:
                    tt_q[1] = s1(1)
                for u in range(nu):
                    # emit_unit first: ACT's copy_A(u) (which frees the PSUM
                    # slot the next PE burst waits on) precedes cvt(u+1),
                    # which isn't consumed until the next iteration.
                    emit_unit(u, hl_q.pop(u))
                    if u + 1 < nu:
                        hl_q[u + 1] = ctt(u + 1, tt_q.pop(u + 1))
                    if u + 2 < nu:
                        tt_q[u + 2] = s1(u + 2)

            if n_loop == 1:
                for _rep in range(n_reps):
                    rep_body()
            else:
                with tc.For_i(0, n_loop, 1):
                    for _rep in range(n_reps):
                        rep_body()
    nc.finalize()
    return nc


_CACHE = {}


def _get_nc(n_reps=1, n_loop=1):
    key = (n_reps, n_loop)
    if key not in _CACHE:
        _CACHE[key] = build(n_reps=n_reps, n_loop=n_loop)
    return _CACHE[key]


def prep_inputs(p_full):
    """p_full [400, 32, 64] f32 (raw [g, (g',c)]) -> per-core in_maps."""
    ayd, axt2 = _make_constants()
    # deinterleave channels: column m = c*32 + g'
    p_d = (
        p_full.reshape(N_SAMPLES, G, G, 2)
        .transpose(0, 1, 3, 2)
        .reshape(N_SAMPLES, G, 2 * G)
    )
    p16 = p_d.astype(np.float16)
    # pack pairs: [npair, 64, 64] with sample 2j on rows 0:32, 2j+1 below
    pp = p16.reshape(N_CORES, S_PER_CORE // 2, 2 * G, 2 * G)
    # host transpose to [core, 64, npair*64] (partition-major)
    pp_t = np.ascontiguousarray(
        pp.transpose(0, 2, 1, 3).reshape(N_CORES, 2 * G, (S_PER_CORE // 2) * 2 * G)
    )
    return [
        {"pp": pp_t[c], "ayd": ayd, "axt2": axt2}
        for c in range(N_CORES)
    ]


def run_on_hw(p_full, n_reps=1):
    """p_full [400, 32, 64] f32 -> out [400, 192, 384] f32."""
    in_maps = prep_inputs(p_full)
    nc = _get_nc(n_reps)
    res = run_bass_kernel_spmd(nc, in_maps, list(range(N_CORES))).results
    out = np.stack([res[c]["out"] for c in range(N_CORES)])
    return out.reshape(N_SAMPLES, H, FW).astype(np.float32)


def kernel(inputs):
    inputs = np.ascontiguousarray(np.asarray(inputs), dtype=np.float32)
    assert inputs.shape == (B, T, 2 * G * G), inputs.shape
    out = run_on_hw(inputs.reshape(N_SAMPLES, G, 2 * G))
    return out.reshape(B, T, H, W, 2)
